# revision 1
# baseline (speedup 1.0000x reference)
"""ConditionalDecoder (GRU seq2seq decoder w/ Bahdanau attention + vocab NLL loss)
on 8 Trainium2 NeuronCores.

Strategy: pure data-parallel over batch B=64 -> 8 rows per core, zero cross-core
communication. Fully-unrolled 99-step recurrence (no For_i barriers) with all
weights SBUF-resident (fp8 e4m3 for the recurrent matrices, bf16 activations;
the tensor engine accepts mixed fp8-stationary x bf16-moving operands). gi0
(input-side GRU0 gates) and ctx_p (attention ctx transform) are precomputed on
the host. The large vocab projection (sumexp over V=32000) is split into 7
waves of <=128 (t,b)-pairs and interleaved into the recurrence as PE gap
filler, which also keeps the PE HAM-warm; only the last waves run as a tail.

Sigmoid is computed as 0.5*(1+tanh(x/2)) with the 0.5 factors folded into
pre-scaled weights, so the only ACT functions used are {Tanh, Exp, Copy} - all
in the single `exp_and_others` table set: no ACT table reloads in steady state.

PSUM accumulation groups are per-bank: gh1 and gi1 target disjoint sub-regions
of one bank with strictly sequential groups; the gh1 half is copied to SBUF
(during the softmax window) before gi1 reuses the bank.

Numerics: fp8 weights / bf16 hidden state & activations, f32 PSUM + f32 exp
accumulation. All biases are zero in this problem (asserted); (b_ih0+b_hh0) is
folded into host-precomputed gi0.
"""
import sys
sys.path.insert(0, '/opt/trn_rl_repo')

import numpy as np
import ml_dtypes

BF16 = ml_dtypes.bfloat16
F8 = ml_dtypes.float8_e4m3fn

T, B, S = 100, 64, 128
E, H, C, V = 512, 1024, 512, 32000
NC = 8                # cores
BL = B // NC          # local batch = 8
NT = T - 1            # 99 steps
TB = NT * BL          # 792 (t,b) pairs per core
TBP = 896             # padded to %128
G3 = 3 * H            # 3072

# wave step boundaries: wave w covers steps WS[w]..WS[w+1]-1 (tb cols 8*WS[w]..)
WS = [0, 16, 32, 48, 64, 80, 96, 99]
NW = len(WS) - 1      # 7 waves
VB = 2000             # vocab cols per wch DMA chunk
NVB = V // VB         # 16 chunks per wave
NVC = 4               # 500-col matmul groups per chunk
VC = VB // NVC        # 500

_cache = {}


def _build_nc():
    import concourse.bacc as bacc
    import concourse.mybir as mybir
    import concourse.tile as tile
    from concourse import masks, tile_utils
    import contextlib

    tile_utils.max_sbuf_usage = 200 * 1024

    f32 = mybir.dt.float32
    bf16 = mybir.dt.bfloat16
    f8 = mybir.dt.float8e4
    AF = mybir.ActivationFunctionType
    AX = mybir.AxisListType

    nc = bacc.Bacc(None, target_bir_lowering=False)

    d_gi0 = nc.dram_tensor("gi0", [128, NT, 24, BL], bf16, kind="ExternalInput")
    d_whh0 = nc.dram_tensor("whh0", [128, 8, G3], f8, kind="ExternalInput")
    d_whh1 = nc.dram_tensor("whh1", [128, 8, G3], f8, kind="ExternalInput")
    d_wih1 = nc.dram_tensor("wih1", [128, 4, G3], f8, kind="ExternalInput")
    d_wh2c = nc.dram_tensor("wh2c", [128, 8, C], f8, kind="ExternalInput")
    d_wh2o = nc.dram_tensor("wh2o", [128, 8, E], bf16, kind="ExternalInput")
    d_wmlp = nc.dram_tensor("wmlp", [128, 4], bf16, kind="ExternalInput")
    d_ctxp = nc.dram_tensor("ctxp", [128, 4, BL, S], bf16, kind="ExternalInput")
    d_ctxZ = nc.dram_tensor("ctxZ", [128, BL, C], f8, kind="ExternalInput")
    d_wrT = nc.dram_tensor("wrT", [128, 4, TBP], bf16, kind="ExternalInput")
    d_wo2p = nc.dram_tensor("wo2p", [128, 4, V], bf16, kind="ExternalInput")
    d_out = nc.dram_tensor("out", [2, TBP], f32, kind="ExternalOutput")

    with tile.TileContext(nc) as tc:
        with contextlib.ExitStack() as octx:
            wp = octx.enter_context(tc.tile_pool(name="w", bufs=1))

            whh0 = wp.tile([128, 8, G3], f8)
            nc.sync.dma_start(whh0[:], d_whh0.ap())
            whh1 = wp.tile([128, 8, G3], f8)
            nc.sync.dma_start(whh1[:], d_whh1.ap())
            wih1 = wp.tile([128, 4, G3], f8)
            nc.sync.dma_start(wih1[:], d_wih1.ap())
            wh2c = wp.tile([128, 8, C], f8)
            nc.sync.dma_start(wh2c[:], d_wh2c.ap())
            wh2o = wp.tile([128, 8, E], bf16)
            nc.sync.dma_start(wh2o[:], d_wh2o.ap())
            wmlp = wp.tile([128, 4], bf16)
            nc.sync.dma_start(wmlp[:], d_wmlp.ap())
            ctxp = wp.tile([128, 4, BL, S], bf16)
            nc.sync.dma_start(ctxp[:], d_ctxp.ap())
            ctxZ = wp.tile([128, BL, C], f8)
            nc.sync.dma_start(ctxZ[:], d_ctxZ.ap())
            wrT = wp.tile([128, 4, TBP], bf16)
            nc.sync.dma_start(wrT[:], d_wrT.ap())

            ones = wp.tile([128, 1], bf16)
            nc.vector.memset(ones[:], 1.0)
            ident = wp.tile([128, 128], bf16)
            masks.make_identity(nc, ident[:])

            o_all = wp.tile([128, 4, TBP], bf16)
            nc.vector.memset(o_all[:], 0.0)
            h2hist = wp.tile([128, 8, TBP], bf16)
            secols = wp.tile([128, NW, NVB * NVC], f32)
            nc.vector.memset(secols[:], 0.0)
            tg_sb = wp.tile([1, TBP], f32)
            nc.vector.memset(tg_sb[:], 0.0)

            lp = octx.enter_context(tc.tile_pool(name="lp", bufs=2))
            gip = octx.enter_context(tc.tile_pool(name="gip", bufs=3))
            wchp = octx.enter_context(tc.tile_pool(name="wch", bufs=2))
            psG = octx.enter_context(tc.tile_pool(name="psG", bufs=2, space="PSUM"))
            psS = octx.enter_context(tc.tile_pool(name="psS", bufs=2, space="PSUM"))
            psSc = octx.enter_context(tc.tile_pool(name="psSc", bufs=1, space="PSUM"))
            psV = octx.enter_context(tc.tile_pool(name="psV", bufs=2, space="PSUM"))

            h2b = lp.tile([128, 8, BL], bf16, tag="h2b")
            nc.vector.memset(h2b[:], 0.0)

            # ---- vocab wave machinery ----
            wch_pending = {}

            def emit_vocab_dma(w, j):
                wch = wchp.tile([128, 4, VB], bf16, tag="wch")
                nc.sync.dma_start(wch[:], d_wo2p.ap()[:, :, j * VB:(j + 1) * VB])
                wch_pending[(w, j)] = wch

            pl_pending = {}

            def emit_vocab_mm(w, j, vc):
                t0, t1 = WS[w], WS[w + 1]
                c0, ncols = 8 * t0, 8 * (t1 - t0)
                wch = wch_pending[(w, j)]
                Pl = psV.tile([128, VC], f32, tag="Pl")
                for co in range(4):
                    nc.tensor.matmul(
                        Pl[0:ncols, :], o_all[:, co, c0:c0 + ncols],
                        wch[:, co, vc * VC:(vc + 1) * VC],
                        start=(co == 0), stop=(co == 3))
                pl_pending[(w, j, vc)] = (Pl, ncols)

            def emit_vocab_exp(w, j, vc):
                Pl, ncols = pl_pending.pop((w, j, vc))
                eb = lp.tile([128, VC], bf16, tag="eb")
                slot = j * NVC + vc
                nc.scalar.activation(
                    eb[0:ncols, :], Pl[0:ncols, :], AF.Exp,
                    accum_out=secols[0:ncols, w, slot:slot + 1])

            def emit_vocab_chunk(w, j, vc):
                emit_vocab_mm(w, j, vc)
                emit_vocab_exp(w, j, vc)

            def emit_wave_head(w):
                t0, t1 = WS[w], WS[w + 1]
                c0, ncols = 8 * t0, 8 * (t1 - t0)
                Po = psS.tile([128, 4, 128], f32, tag="S")
                for mc in range(4):
                    for kc in range(8):
                        nc.tensor.matmul(
                            Po[:, mc, 0:ncols], wh2o[:, kc, mc * 128:(mc + 1) * 128],
                            h2hist[:, kc, c0:c0 + ncols],
                            start=(kc == 0), stop=(kc == 7))
                nc.scalar.activation(
                    o_all[:, :, c0:c0 + ncols], Po[:, :, 0:ncols], AF.Tanh)
                prod = lp.tile([128, 4, 128], bf16, tag="prod")
                nc.vector.tensor_mul(
                    prod[:, :, 0:ncols], wrT[:, :, c0:c0 + ncols],
                    o_all[:, :, c0:c0 + ncols])
                Pt = psS.tile([1, 128], f32, tag="S")
                for co in range(4):
                    nc.tensor.matmul(Pt[:, 0:ncols], ones[:], prod[:, co, 0:ncols],
                                     start=(co == 0), stop=(co == 3))
                nc.scalar.activation(tg_sb[:, c0:c0 + ncols], Pt[:, 0:ncols], AF.Copy)

            # vocab emission schedule: wave w's 16 chunks spread over the steps
            # following its completion (chunks landing past the loop go to tail).
            sched = {}
            for w in range(NW):
                for j in range(NVB):
                    st = WS[w + 1] + j
                    sched.setdefault(min(st, NT), []).append((w, j))

            # gi0 streaming (one [128,24,8] slab per step, prefetched 2 ahead)
            gi0_tiles = {}

            def emit_gi0_dma(t):
                if t < NT:
                    g = gip.tile([128, 24, BL], bf16, tag="gi0")
                    nc.sync.dma_start(g[:], d_gi0.ap()[:, t])
                    gi0_tiles[t] = g

            emit_gi0_dma(0)
            emit_gi0_dma(1)

            # ---------------- the recurrence, fully unrolled ----------------
            for t in range(NT):
                emit_gi0_dma(t + 2)
                if t + 1 < NT:
                    for item in sched.get(t + 1, []):
                        emit_vocab_dma(*item)
                cur = sched.get(t, [])

                # -- gh0 = W_hh0' @ h2(t-1): rz then n (all rows pre-halved) --
                P0 = psG.tile([128, 24, BL], f32, tag="G")
                for mc in range(24):
                    for kc in range(8):
                        nc.tensor.matmul(
                            P0[:, mc, :], whh0[:, kc, mc * 128:(mc + 1) * 128],
                            h2b[:, kc, :], start=(kc == 0), stop=(kc == 7))
                gi0t = gi0_tiles.pop(t)
                # -- GRU0 elementwise (sigmoid-free) --
                xrz = lp.tile([128, 16, BL], bf16, tag="xrz")
                nc.vector.tensor_add(
                    xrz[:].rearrange("p m b -> p (m b)"),
                    gi0t[:, 0:16, :].rearrange("p m b -> p (m b)"),
                    P0[:, 0:16, :].rearrange("p m b -> p (m b)"))
                trz = lp.tile([128, 16, BL], bf16, tag="trz")
                nc.scalar.activation(trz[:].rearrange("p m b -> p (m b)"),
                                     xrz[:].rearrange("p m b -> p (m b)"), AF.Tanh)
                q = lp.tile([128, 8, BL], bf16, tag="q")
                nc.vector.tensor_mul(
                    q[:].rearrange("p m b -> p (m b)"),
                    trz[:, 0:8, :].rearrange("p m b -> p (m b)"),
                    P0[:, 16:24, :].rearrange("p m b -> p (m b)"))
                a1 = lp.tile([128, 8, BL], bf16, tag="a1")
                nc.vector.tensor_add(
                    a1[:].rearrange("p m b -> p (m b)"),
                    gi0t[:, 16:24, :].rearrange("p m b -> p (m b)"),
                    P0[:, 16:24, :].rearrange("p m b -> p (m b)"))
                nin = lp.tile([128, 8, BL], bf16, tag="nin")
                nc.vector.tensor_add(nin[:].rearrange("p m b -> p (m b)"),
                                     a1[:].rearrange("p m b -> p (m b)"),
                                     q[:].rearrange("p m b -> p (m b)"))
                nt0 = lp.tile([128, 8, BL], bf16, tag="nt0")
                nc.scalar.activation(nt0[:].rearrange("p m b -> p (m b)"),
                                     nin[:].rearrange("p m b -> p (m b)"), AF.Tanh)
                dd = lp.tile([128, 8, BL], bf16, tag="dd")
                nc.vector.tensor_sub(dd[:].rearrange("p m b -> p (m b)"),
                                     h2b[:].rearrange("p m b -> p (m b)"),
                                     nt0[:].rearrange("p m b -> p (m b)"))
                ee = lp.tile([128, 8, BL], bf16, tag="ee")
                nc.vector.tensor_mul(ee[:].rearrange("p m b -> p (m b)"),
                                     trz[:, 8:16, :].rearrange("p m b -> p (m b)"),
                                     dd[:].rearrange("p m b -> p (m b)"))
                ss = lp.tile([128, 8, BL], bf16, tag="ss")
                nc.vector.tensor_add(ss[:].rearrange("p m b -> p (m b)"),
                                     dd[:].rearrange("p m b -> p (m b)"),
                                     ee[:].rearrange("p m b -> p (m b)"))
                s2 = lp.tile([128, 8, BL], bf16, tag="s2")
                nc.vector.tensor_scalar_mul(s2[:].rearrange("p m b -> p (m b)"),
                                            ss[:].rearrange("p m b -> p (m b)"), 0.5)
                h1b = lp.tile([128, 8, BL], bf16, tag="h1b")
                nc.vector.tensor_add(h1b[:].rearrange("p m b -> p (m b)"),
                                     nt0[:].rearrange("p m b -> p (m b)"),
                                     s2[:].rearrange("p m b -> p (m b)"))

                # -- hid = W_h2c @ h1 --
                Ph = psS.tile([128, 4, BL], f32, tag="S")
                for mc in range(4):
                    for kc in range(8):
                        nc.tensor.matmul(
                            Ph[:, mc, :], wh2c[:, kc, mc * 128:(mc + 1) * 128],
                            h1b[:, kc, :], start=(kc == 0), stop=(kc == 7))
                hidb = lp.tile([128, 4, BL], bf16, tag="hidb")
                nc.vector.tensor_copy(hidb[:].rearrange("p m b -> p (m b)"),
                                      Ph[:].rearrange("p m b -> p (m b)"))

                # wave head (Po + tg) and vocab fillers land here: their PE work
                # fills the attention window while DVE/ACT run the tanh chain
                for w in range(NW):
                    if WS[w + 1] == t:
                        emit_wave_head(w)
                if cur:
                    emit_vocab_mm(cur[0][0], cur[0][1], 0)

                # -- attention scores; gh1 interleaved as PE work --
                # P1 bank layout: [0:16] gh1 rz', [16:24] gh1 n', [24:40] gi1 rz',
                #                 [40:48] gi1 n   (groups strictly sequential)
                Sc = psSc.tile([1, BL * S], f32, tag="Sc")
                P1 = psG.tile([128, 48, BL], f32, tag="G")
                for co in range(4):
                    u = lp.tile([128, BL, S], bf16, tag="u")
                    nc.vector.tensor_add(
                        u[:], ctxp[:, co],
                        hidb[:, co, :].to_broadcast((128, BL, S)))
                    th = lp.tile([128, BL, S], bf16, tag="th")
                    nc.scalar.activation(th[:], u[:], AF.Tanh)
                    thf = th[:].rearrange("p b s -> p (b s)")
                    for nn in range(2):
                        nc.tensor.matmul(
                            Sc[:, nn * 512:(nn + 1) * 512], wmlp[:, co:co + 1],
                            thf[:, nn * 512:(nn + 1) * 512],
                            start=(co == 0), stop=(co == 3))
                    for mc in range(co * 6, min(co * 6 + 6, 24)):
                        for kc in range(8):
                            nc.tensor.matmul(
                                P1[:, mc, :], whh1[:, kc, mc * 128:(mc + 1) * 128],
                                h1b[:, kc, :], start=(kc == 0), stop=(kc == 7))
                    if co == 0 and cur:
                        emit_vocab_mm(cur[0][0], cur[0][1], 1)
                # vocab exps drain here: ACT is past the tanh chain, waiting on
                # the scb DMA round-trip
                if cur:
                    emit_vocab_exp(cur[0][0], cur[0][1], 0)
                    emit_vocab_exp(cur[0][0], cur[0][1], 1)
                # gh1 -> SBUF so the bank can host gi1's groups
                gh1sb = lp.tile([128, 24, BL], bf16, tag="gh1sb")
                nc.vector.tensor_copy(gh1sb[:].rearrange("p m b -> p (m b)"),
                                      P1[:, 0:24, :].rearrange("p m b -> p (m b)"))

                # -- softmax over s --
                scs = lp.tile([1, BL * S], bf16, tag="scs")
                nc.vector.tensor_copy(scs[:], Sc[:])
                scb = lp.tile([BL, S], bf16, tag="scb")
                nc.sync.dma_start(scb[:], scs[:].rearrange("o (b s) -> o b s", b=BL))
                # vc2's matmuls keep PE fed while the softmax chain runs
                if cur:
                    emit_vocab_mm(cur[0][0], cur[0][1], 2)
                Ee = lp.tile([BL, S], f32, tag="Ee")
                nc.scalar.activation(Ee[:], scb[:], AF.Exp)
                if cur:
                    emit_vocab_exp(cur[0][0], cur[0][1], 2)
                Dd = lp.tile([BL, 1], f32, tag="Dd")
                nc.vector.reduce_sum(Dd[:], Ee[:], axis=AX.X)
                rD = lp.tile([BL, 1], f32, tag="rD")
                nc.vector.reciprocal(rD[:], Dd[:])
                al = lp.tile([BL, S], bf16, tag="al")
                nc.vector.tensor_scalar_mul(al[:], Ee[:], rD[:])
                alT = psS.tile([128, BL], bf16, tag="S")
                nc.tensor.transpose(alT[:], al[:], ident[0:BL, 0:BL])
                alTs = lp.tile([128, BL], bf16, tag="alTs")
                nc.vector.tensor_copy(alTs[:], alT[:])

                # -- z = sum_s alpha * ctx --
                Pz = psS.tile([128, 4, BL], f32, tag="S")
                for b in range(BL):
                    for cc in range(4):
                        nc.tensor.matmul(
                            Pz[:, cc, b:b + 1],
                            ctxZ[:, b, cc * 128:(cc + 1) * 128],
                            alTs[:, b:b + 1], start=True, stop=True)
                zb = lp.tile([128, 4, BL], bf16, tag="zb")
                nc.vector.tensor_copy(zb[:].rearrange("p m b -> p (m b)"),
                                      Pz[:].rearrange("p m b -> p (m b)"))

                # -- gi1 (rz' rows pre-halved) into P1[24:48] --
                for mc in range(16):
                    for kc in range(4):
                        nc.tensor.matmul(
                            P1[:, 24 + mc, :], wih1[:, kc, mc * 128:(mc + 1) * 128],
                            zb[:, kc, :], start=(kc == 0), stop=(kc == 3))
                for mc in range(8):
                    for kc in range(4):
                        nc.tensor.matmul(
                            P1[:, 40 + mc, :],
                            wih1[:, kc, (16 + mc) * 128:(17 + mc) * 128],
                            zb[:, kc, :], start=(kc == 0), stop=(kc == 3))
                # vc3's matmuls keep PE fed while the GRU1 chain runs
                if cur:
                    emit_vocab_mm(cur[0][0], cur[0][1], 3)
                for w2, j2 in cur[1:]:
                    emit_vocab_mm(w2, j2, 0)
                    emit_vocab_exp(w2, j2, 0)
                    emit_vocab_mm(w2, j2, 1)
                    emit_vocab_exp(w2, j2, 1)

                # -- GRU1 elementwise --
                xrz1 = lp.tile([128, 16, BL], bf16, tag="xrz1")
                nc.vector.tensor_add(
                    xrz1[:].rearrange("p m b -> p (m b)"),
                    gh1sb[:, 0:16, :].rearrange("p m b -> p (m b)"),
                    P1[:, 24:40, :].rearrange("p m b -> p (m b)"))
                trz1 = lp.tile([128, 16, BL], bf16, tag="trz1")
                nc.scalar.activation(trz1[:].rearrange("p m b -> p (m b)"),
                                     xrz1[:].rearrange("p m b -> p (m b)"), AF.Tanh)
                q1 = lp.tile([128, 8, BL], bf16, tag="q1")
                nc.vector.tensor_mul(
                    q1[:].rearrange("p m b -> p (m b)"),
                    trz1[:, 0:8, :].rearrange("p m b -> p (m b)"),
                    gh1sb[:, 16:24, :].rearrange("p m b -> p (m b)"))
                b1 = lp.tile([128, 8, BL], bf16, tag="b1")
                nc.vector.tensor_add(
                    b1[:].rearrange("p m b -> p (m b)"),
                    P1[:, 40:48, :].rearrange("p m b -> p (m b)"),
                    q1[:].rearrange("p m b -> p (m b)"))
                b2 = lp.tile([128, 8, BL], bf16, tag="b2")
                nc.vector.tensor_add(
                    b2[:].rearrange("p m b -> p (m b)"),
                    b1[:].rearrange("p m b -> p (m b)"),
                    gh1sb[:, 16:24, :].rearrange("p m b -> p (m b)"))
                nt1 = lp.tile([128, 8, BL], bf16, tag="nt1")
                nc.scalar.activation(nt1[:].rearrange("p m b -> p (m b)"),
                                     b2[:].rearrange("p m b -> p (m b)"), AF.Tanh)
                dd1 = lp.tile([128, 8, BL], bf16, tag="dd1")
                nc.vector.tensor_sub(dd1[:].rearrange("p m b -> p (m b)"),
                                     h1b[:].rearrange("p m b -> p (m b)"),
                                     nt1[:].rearrange("p m b -> p (m b)"))
                e1 = lp.tile([128, 8, BL], bf16, tag="e1")
                nc.vector.tensor_mul(e1[:].rearrange("p m b -> p (m b)"),
                                     trz1[:, 8:16, :].rearrange("p m b -> p (m b)"),
                                     dd1[:].rearrange("p m b -> p (m b)"))
                s1 = lp.tile([128, 8, BL], bf16, tag="s1")
                nc.vector.tensor_add(s1[:].rearrange("p m b -> p (m b)"),
                                     dd1[:].rearrange("p m b -> p (m b)"),
                                     e1[:].rearrange("p m b -> p (m b)"))
                s1h = lp.tile([128, 8, BL], bf16, tag="s1h")
                nc.vector.tensor_scalar_mul(s1h[:].rearrange("p m b -> p (m b)"),
                                            s1[:].rearrange("p m b -> p (m b)"), 0.5)
                h2b = lp.tile([128, 8, BL], bf16, tag="h2b")
                nc.vector.tensor_add(h2b[:].rearrange("p m b -> p (m b)"),
                                     nt1[:].rearrange("p m b -> p (m b)"),
                                     s1h[:].rearrange("p m b -> p (m b)"))
                nc.vector.tensor_copy(h2hist[:, :, t * BL:(t + 1) * BL], h2b[:])
                if cur:
                    emit_vocab_exp(cur[0][0], cur[0][1], 3)
                for w2, j2 in cur[1:]:
                    emit_vocab_mm(w2, j2, 2)
                    emit_vocab_exp(w2, j2, 2)
                    emit_vocab_mm(w2, j2, 3)
                    emit_vocab_exp(w2, j2, 3)

            # ---------------- tail: last waves + final reduction ----------------
            for w in range(NW):
                if WS[w + 1] == NT:
                    emit_wave_head(w)
            for w2, j2 in sched.get(NT, []):
                if (w2, j2) not in wch_pending:
                    emit_vocab_dma(w2, j2)
                for vc in range(NVC):
                    emit_vocab_chunk(w2, j2, vc)

            se = wp.tile([128, NW], f32)
            nc.vector.reduce_sum(se[:], secols[:], axis=AX.X)
            sesb = wp.tile([1, TBP], f32)
            nc.sync.dma_start(sesb[:].rearrange("o (c p) -> o c p", p=128), se[:])
            nc.sync.dma_start(d_out.ap()[0:1, :], sesb[:])
            nc.sync.dma_start(d_out.ap()[1:2, :], tg_sb[:])

    nc.finalize()
    return nc


def _prep_inputs(y, ctx, emb, W_ih0, W_hh0, b_ih0, b_hh0, W_ih1, W_hh1, b_ih1, b_hh1,
                 W_c2c, W_h2c, w_mlp, W_h2o, b_h2o, W_o2p, b_o2p):
    f = np.float32
    y = np.asarray(y)
    ctx = np.asarray(ctx, f)
    emb = np.asarray(emb, f)
    W_ih0, W_hh0 = np.asarray(W_ih0, f), np.asarray(W_hh0, f)
    W_ih1, W_hh1 = np.asarray(W_ih1, f), np.asarray(W_hh1, f)
    b_ih0, b_hh0 = np.asarray(b_ih0, f), np.asarray(b_hh0, f)
    W_c2c, W_h2c = np.asarray(W_c2c, f), np.asarray(W_h2c, f)
    w_mlp, W_h2o = np.asarray(w_mlp, f), np.asarray(W_h2o, f)
    b_h2o = np.asarray(b_h2o, f)
    W_o2p, b_o2p = np.asarray(W_o2p, f), np.asarray(b_o2p, f)

    # The tanh-sigmoid identity folds 0.5 into rz rows; the hh-side n-row 0.5
    # implements r*gh_n = P0n' + tau_r*P0n'. Requires these biases to be zero:
    assert abs(b_hh0[2 * H:]).max() == 0 and abs(b_hh1).max() == 0
    assert abs(b_ih1).max() == 0 and abs(b_h2o).max() == 0 and abs(b_o2p).max() == 0

    rzh = np.ones((G3,), f)
    rzh[:2 * H] = 0.5

    def to8(x):
        return np.ascontiguousarray(np.asarray(x, f).astype(F8))

    def tob(x):
        return np.ascontiguousarray(np.asarray(x, f).astype(BF16))

    # gi0 (host): y_emb @ (rz-halved W_ih0).T + scaled bias -> [NT, B, G3]
    gi0_full = (emb[y[:NT]].reshape(-1, E) @ (W_ih0 * rzh[:, None]).T
                + (b_ih0 + b_hh0) * rzh).reshape(NT, B, G3)

    common = dict(
        whh0=to8(np.transpose((0.5 * W_hh0).T.reshape(8, 128, G3), (1, 0, 2))),
        whh1=to8(np.transpose((0.5 * W_hh1).T.reshape(8, 128, G3), (1, 0, 2))),
        wih1=to8(np.transpose((W_ih1 * rzh[:, None]).T.reshape(4, 128, G3), (1, 0, 2))),
        wh2c=to8(np.transpose(W_h2c.T.reshape(8, 128, C), (1, 0, 2))),
        wh2o=tob(np.transpose(W_h2o.T.reshape(8, 128, E), (1, 0, 2))),
        wmlp=tob(w_mlp.reshape(4, 128).T),
        wo2p=tob(np.transpose(W_o2p.T.reshape(4, 128, V), (1, 0, 2))),
    )

    ctx_p = np.einsum('sbc,kc->sbk', ctx, W_c2c)  # (S,B,C)
    wo2p_b = W_o2p.astype(BF16)

    def rowsT(ids):
        g = np.zeros((TBP, E), BF16)
        g[:len(ids)] = wo2p_b[ids]
        return np.ascontiguousarray(np.transpose(g.reshape(TBP, 4, 128), (2, 1, 0)))

    in_maps = []
    for qq in range(NC):
        bq = slice(qq * BL, (qq + 1) * BL)
        gi0_l = np.transpose(
            gi0_full[:, bq, :].reshape(NT, BL, 24, 128), (3, 0, 2, 1))
        cq = ctx[:, bq, :]
        m = dict(common)
        m.update(
            gi0=np.ascontiguousarray(gi0_l.astype(BF16)),
            ctxp=tob(np.transpose(ctx_p[:, bq, :].reshape(S, BL, 4, 128),
                                  (3, 2, 1, 0))),
            ctxZ=to8(cq),
            wrT=rowsT(np.asarray(y[1:, bq]).reshape(-1)),
        )
        in_maps.append(m)
    return in_maps


def kernel(**inputs):
    from concourse import bass_utils
    if 'nc' not in _cache:
        _cache['nc'] = _build_nc()
    nc = _cache['nc']
    in_maps = _prep_inputs(**inputs)
    res = bass_utils.run_bass_kernel_spmd(nc, in_maps, core_ids=list(range(NC)))
    _cache['last_res'] = res

    y = np.asarray(inputs['y'])
    total = np.float64(0.0)
    for qq in range(NC):
        out = res.results[qq]["out"]  # (2, TBP)
        # the se DMA streams the [128, NW] tile row-major: flat = p*NW + w;
        # decode to tb = w*128 + p ordering
        se = out[0].reshape(128, NW).T.reshape(-1).astype(np.float64)
        tgt = out[1].astype(np.float64)
        y_next = y[1:, qq * BL:(qq + 1) * BL].reshape(-1)  # (TB,) t-major
        mask = (y_next != 0)
        total += np.sum(np.where(mask, np.log(se[:TB]) - tgt[:TB], 0.0))
    return np.float32(total)

